# revision 24
# baseline (speedup 1.0000x reference)
"""Gated axial attention (width axis) Trainium2 Bass kernel.

Sharding: data-parallel over the fused B*H row axis (512 rows -> 64 rows
per core on 8 cores). Each core computes full attention for its rows;
no collectives. All matmuls run in bf16 with fp32 PSUM accumulation.

Optimizations vs the straightforward version:
  * scores via s^T = (k+pq')^T (q'+pk') - pq'^T pk': 2 matmuls per
    (row, head) instead of 3. The (k+pq')/(q'+pk') sums are formed by
    the PSUM->SBUF drain on DVE (tensor_add with a replicated pos+bias
    table), replacing the ACT bias-copy; the -pq'^T pk' term is a
    matmul of two constant tiles.
  * score matmuls are K=64 (half the PE array). Adjacent matmuls
    alternate head parity (base partitions 0-63 / 64-127 -> PE row
    tiles (0,0) / (64,0)) so consecutive matmuls execute concurrently
    in the two array halves (~2x). Both parities accumulate in ONE
    2-bank psum tile; since start=True clears has_written for a whole
    bank, only the first matmul into each bank carries start=True.
  * one exp ACT per row over both banks; softmax normalize fused into
    reciprocal_approx_fast + one broadcast tensor_mul per (row, 4-head
    group) instead of 4 tensor_scalar_muls.
  * const DMA loads split across the gpsimd+scalar queues and ordered
    so the first projection drains don't queue behind 3MB of weights.

Layouts per core (P = 128 partitions):
  xt    [64, 512, 128]  per-row X^T = [c_in, w] slices (native x layout)
  qpk/kpq computed as [c_out, tok] (weights stationary, tokens moving)
  V computed as [tok, c_out] (X^T stationary, weight moving)
  scores computed transposed: sT[j, i] per head via 2 accumulated matmuls
  exp on ACT -> bf16; AV matmul with expT stationary -> out [i, d] plus a
  ones-column in vmix producing the softmax denominator per partition i;
  normalize on DVE; PE-transpose attn-out back to [c, tok]; o-projection.

Scale folding (host side): 1/sqrt(hd)=0.125 into q_w/q_b; g_q into pos_q;
g_k*scale into pos_k (negated copy for the correction term); g_v1 into
v_w and v_b; g_v2 into pos_v; o_b added on host after gathering.
"""

import sys
import types

sys.path.insert(0, "/opt/trn_rl_repo")


def _install_ntff_shim():
    """Make bass_utils trace=True work under axon (BASS_TRACE=1)."""
    try:
        import antenv
    except ImportError:
        return
    if "antenv.axon_hooks" in sys.modules:
        return
    mod = types.ModuleType("antenv.axon_hooks")
    _hook = [None]

    def set_axon_ntff_profile_hook(h):
        _hook[0] = h

    def get_axon_ntff_profile_hook():
        if _hook[0] is None:
            try:
                if "/root/.axon_site" not in sys.path:
                    sys.path.insert(0, "/root/.axon_site")
                from trn_agent_boot.trn_boot import _ntff_profile_via_ctypes

                _hook[0] = _ntff_profile_via_ctypes("/opt/axon/libaxon_pjrt.so")
            except Exception:
                _hook[0] = None
        return _hook[0]

    mod.set_axon_ntff_profile_hook = set_axon_ntff_profile_hook
    mod.get_axon_ntff_profile_hook = get_axon_ntff_profile_hook
    sys.modules["antenv.axon_hooks"] = mod
    antenv.axon_hooks = mod


_install_ntff_shim()

import ml_dtypes  # noqa: E402
import numpy as np  # noqa: E402

import concourse.bass as bass  # noqa: E402
import concourse.tile as tile  # noqa: E402
from concourse import bacc, mybir  # noqa: E402
from concourse.bass_utils import run_bass_kernel_spmd  # noqa: E402

BF16 = ml_dtypes.bfloat16

B, C, H, W = 4, 512, 128, 128
NH, HD = 8, 64
NCORES = 8
ROWS = B * H  # 512 fused rows
RPC = ROWS // NCORES  # 64 rows per core
BLK = 16  # rows per block
NBLK = RPC // BLK
P = 128
NCH = C // P  # 4 channel chunks of 128
TOK = BLK * W  # tokens per block (2048)
NTT = TOK // 512  # 512-token tiles per block

_CACHED_NC = None
LAST_RESULTS = None


def _build_nc():
    nc = bacc.Bacc("TRN2", target_bir_lowering=False, debug=False,
                   num_devices=NCORES)
    dt = mybir.dt

    xt = nc.dram_tensor("xt", [NCH, P, RPC, W], dt.bfloat16,
                        kind="ExternalInput")
    q_wt = nc.dram_tensor("q_wt", [C, C], dt.bfloat16, kind="ExternalInput")
    k_wt = nc.dram_tensor("k_wt", [C, C], dt.bfloat16, kind="ExternalInput")
    v_wt = nc.dram_tensor("v_wt", [C, C], dt.bfloat16, kind="ExternalInput")
    o_wt = nc.dram_tensor("o_wt", [C, C], dt.bfloat16, kind="ExternalInput")
    pqts = nc.dram_tensor("pqts", [NCH, P, W], dt.bfloat16, kind="ExternalInput")
    mpkts = nc.dram_tensor("mpkts", [NCH, P, W], dt.bfloat16,
                           kind="ExternalInput")
    qpk_rep = nc.dram_tensor("qpk_rep", [NCH, P, 512], dt.bfloat16,
                             kind="ExternalInput")
    kpq_rep = nc.dram_tensor("kpq_rep", [NCH, P, 512], dt.bfloat16,
                             kind="ExternalInput")
    pvs = nc.dram_tensor("pvs", [W, C], dt.bfloat16, kind="ExternalInput")
    ident = nc.dram_tensor("ident", [P, P], dt.bfloat16, kind="ExternalInput")
    out_t = nc.dram_tensor("out_t", [NCH, P, RPC, W], dt.float32,
                           kind="ExternalOutput")

    with tile.TileContext(nc) as tc:
        with (
            tc.tile_pool(name="const", bufs=1) as const,
            tc.tile_pool(name="xtp", bufs=2) as xtp,
            tc.tile_pool(name="big", bufs=1) as big,
            tc.tile_pool(name="expp", bufs=1) as expp,
            tc.tile_pool(name="small", bufs=4) as small,
            tc.tile_pool(name="fop", bufs=3) as fop,
            tc.tile_pool(name="ps_proj", bufs=2, space="PSUM") as ps_proj,
            tc.tile_pool(name="ps_sc", bufs=1, space="PSUM") as ps_sc,
            tc.tile_pool(name="ps_av", bufs=2, space="PSUM") as ps_av,
            tc.tile_pool(name="ps_tr", bufs=2, space="PSUM") as ps_tr,
        ):
            # ---- constants into SBUF ----
            # Interleave across the gpsimd + scalar DMA queues, ordered so
            # the tiles needed by the earliest compute (qw + its drain
            # table) land first; otherwise the first PSUM drains stall
            # behind 3MB of weight DMAs and the PE idles ~13us at start.
            def load_w(name, dram, eng):
                t = const.tile([P, NCH, C], dt.bfloat16, name=name)
                src = dram.ap().rearrange("(k p) c -> p k c", p=P)
                for k in range(NCH):
                    eng.dma_start(out=t[:, k, :], in_=src[:, k, :])
                return t

            qw_sb = load_w("qw_sb", q_wt, nc.gpsimd)
            qpkr_sb = const.tile([P, NCH, 512], dt.bfloat16)
            nc.scalar.dma_start(out=qpkr_sb,
                              in_=qpk_rep.ap().rearrange("k p w -> p k w"))
            kw_sb = load_w("kw_sb", k_wt, nc.scalar)
            kpqr_sb = const.tile([P, NCH, 512], dt.bfloat16)
            nc.gpsimd.dma_start(out=kpqr_sb,
                              in_=kpq_rep.ap().rearrange("k p w -> p k w"))
            vw_sb = load_w("vw_sb", v_wt, nc.gpsimd)
            pv_sb = const.tile([P, C], dt.bfloat16)
            nc.scalar.dma_start(out=pv_sb, in_=pvs.ap())
            pq_sb = const.tile([P, NCH, W], dt.bfloat16)
            nc.scalar.dma_start(out=pq_sb,
                              in_=pqts.ap().rearrange("k p w -> p k w"))
            mpk_sb = const.tile([P, NCH, W], dt.bfloat16)
            nc.gpsimd.dma_start(out=mpk_sb,
                              in_=mpkts.ap().rearrange("k p w -> p k w"))
            ow_sb = load_w("ow_sb", o_wt, nc.scalar)
            id_sb = const.tile([P, P], dt.bfloat16)
            nc.gpsimd.dma_start(out=id_sb, in_=ident.ap())

            xt_r = xt.ap()
            out_r = out_t.ap()

            for blk in range(NBLK):
                r0 = blk * BLK

                # ---- load X^T block: [P, NCH, BLK*W] ----
                # 4 slices per chunk so the first projection matmuls can
                # start before the whole block has landed.
                xt_sb = xtp.tile([P, NCH, TOK], dt.bfloat16, tag="xt")
                for nt in range(NTT):
                    for k in range(NCH):
                        nc.sync.dma_start(
                            out=xt_sb[:, k, nt * 512:(nt + 1) * 512],
                            in_=xt_r[k, :, r0 + nt * 4:r0 + nt * 4 + 4, :]
                            .rearrange("p r w -> p (r w)"))

                # ---- q'+pk' and k+pq' projections: [c_out, tok] ----
                # PSUM->SBUF drain on DVE adds the replicated pos+bias
                # table, so the score matmul needs only 2 terms.
                qpk_sb = big.tile([P, NCH, TOK], dt.bfloat16, tag="qt")
                kpq_sb = big.tile([P, NCH, TOK], dt.bfloat16, tag="kt")
                for wsb, rep, dst in ((qw_sb, qpkr_sb, qpk_sb),
                                      (kw_sb, kpqr_sb, kpq_sb)):
                    for m in range(NCH):
                        for n in range(NTT):
                            ps = ps_proj.tile([P, 512], dt.float32, tag="pp",
                                              name="ps")
                            for k in range(NCH):
                                nc.tensor.matmul(
                                    ps,
                                    lhsT=wsb[:, k, m * P:(m + 1) * P],
                                    rhs=xt_sb[:, k, n * 512:(n + 1) * 512],
                                    start=(k == 0), stop=(k == NCH - 1))
                            nc.vector.tensor_add(
                                out=dst[:, m, n * 512:(n + 1) * 512],
                                in0=ps, in1=rep[:, m, :])

                # ---- V projection ([tok, c]) + vmix build ----
                vmix = big.tile([P, BLK, NH * 65], dt.bfloat16, tag="vmix")
                # ones columns for the softmax denominator
                nc.vector.memset(
                    vmix.rearrange("p r (h e) -> p r h e", e=65)
                    [:, :, :, 64:65], 1.0)
                for r in range(BLK):
                    ps = ps_proj.tile([P, 512], dt.float32, tag="pp",
                                      name="ps")
                    for k in range(NCH):
                        nc.tensor.matmul(
                            ps,
                            lhsT=xt_sb[:, k, r * P:(r + 1) * P],
                            rhs=vw_sb[:, k, :],
                            start=(k == 0), stop=(k == NCH - 1))
                    nc.vector.tensor_add(
                        out=vmix[:, r, :].rearrange(
                            "p (h e) -> p h e", e=65)[:, :, 0:64],
                        in0=ps.rearrange("p (h e) -> p h e", e=64),
                        in1=pv_sb.rearrange("p (h e) -> p h e", e=64))

                # ---- scores + exp, parity-split psum ----
                # sT[j, i] = kpq_j . qpk_i - pq'_j . pk'_i  per head
                exp_all = expp.tile([P, BLK, 2, 512], dt.bfloat16, tag="exp")
                for r in range(BLK):
                    # One 2-bank psum tile: even heads in bank half 0
                    # (operands at partitions 0-63, PE row-tile (0,0)), odd
                    # heads in half 1 (partitions 64-127, row-tile (64,0)).
                    # Adjacent matmuls alternate parity, so they overlap in
                    # the PE array (~2x on this K=64 phase); the single
                    # tile keeps the scheduler from re-serializing the
                    # stream into per-parity chains.
                    # HW: start=True clears has_written for the WHOLE bank,
                    # so only the first matmul into each bank carries it.
                    # Later var matmuls (start=False) overwrite their still
                    # -clear slices; the deferred constant -pq'.pk' matmuls
                    # then accumulate onto set bits.
                    psc = ps_sc.tile([P, 2, 512], dt.float32, tag="sc",
                                     name="psc")
                    for h in range(NH):
                        par = h % 2
                        ch = h // 2
                        lo = par * HD
                        idx = h // 2
                        nc.tensor.matmul(
                            psc[:, par, idx * P:(idx + 1) * P],
                            lhsT=kpq_sb[lo:lo + HD, ch, r * P:(r + 1) * P],
                            rhs=qpk_sb[lo:lo + HD, ch, r * P:(r + 1) * P],
                            start=(h < 2), stop=False, skip_group_check=True)
                    for h in range(NH):
                        par = h % 2
                        ch = h // 2
                        lo = par * HD
                        idx = h // 2
                        nc.tensor.matmul(
                            psc[:, par, idx * P:(idx + 1) * P],
                            lhsT=pq_sb[lo:lo + HD, ch, :],
                            rhs=mpk_sb[lo:lo + HD, ch, :],
                            start=False, stop=(h >= NH - 2),
                            skip_group_check=True)
                    nc.scalar.activation(
                        exp_all[:, r, :, :], psc,
                        mybir.ActivationFunctionType.Exp)

                # ---- AV + fused normalize ----
                ao_sb = big.tile([P, BLK, C], dt.bfloat16, tag="ao")
                for r in range(BLK):
                    for hg in range(2):
                        psa = ps_av.tile([P, 4 * 65], dt.float32, tag="av",
                                         name="psa")
                        for hh in range(4):
                            h = hg * 4 + hh
                            par = h % 2
                            idx = h // 2
                            nc.tensor.matmul(
                                psa[:, hh * 65:(hh + 1) * 65],
                                lhsT=exp_all[:, r, par,
                                             idx * P:(idx + 1) * P],
                                rhs=vmix[:, r, h * 65:(h + 1) * 65],
                                start=True, stop=True)
                        # fast approximate reciprocal (~18 bits, plenty for
                        # a softmax denominator feeding bf16)
                        rv = small.tile([P, 4, 1], dt.float32, tag="rv",
                                        name="rv")
                        nc.vector.reciprocal_approx_fast(
                            rv,
                            psa.rearrange("p (h e) -> p h e", e=65)
                            [:, :, 64:65])
                        nc.vector.tensor_mul(
                            out=ao_sb[:, r, hg * 256:(hg + 1) * 256]
                            .rearrange("p (h e) -> p h e", e=64),
                            in0=psa.rearrange("p (h e) -> p h e", e=65)
                            [:, :, 0:64],
                            in1=rv.broadcast_to([P, 4, 64]))

                # ---- transpose attn-out + output projection, interleaved
                # per 4-row group so o-proj matmuls overlap the DVE
                # copybacks of the next group's transposes ----
                aot_sb = big.tile([P, NCH, TOK], dt.bfloat16, tag="aot")
                for n in range(NTT):
                    for rr in range(4):
                        r = n * 4 + rr
                        pst = ps_tr.tile([P, NCH, P], dt.bfloat16, tag="tr",
                                         name="pst")
                        for ch in range(NCH):
                            nc.tensor.transpose(
                                pst[:, ch, :],
                                ao_sb[:, r, ch * P:(ch + 1) * P], id_sb)
                        # one batched copyback per row: [128, 4, 128].
                        # On ACT: the DVE queue in this window is busy with
                        # the next block's projection drains, which stalled
                        # the transpose stream ~700ns per group.
                        nc.scalar.copy(
                            aot_sb[:, :, r * P:(r + 1) * P], pst)
                    for m in range(NCH):
                        ps = ps_proj.tile([P, 512], dt.float32, tag="pp",
                                          name="ps")
                        for k in range(NCH):
                            nc.tensor.matmul(
                                ps,
                                lhsT=ow_sb[:, k, m * P:(m + 1) * P],
                                rhs=aot_sb[:, k, n * 512:(n + 1) * 512],
                                start=(k == 0), stop=(k == NCH - 1))
                        fo = fop.tile([P, 512], dt.float32, tag="fo",
                                      name="fo")
                        nc.scalar.copy(fo, ps)
                        nc.gpsimd.dma_start(
                            out=out_r[m, :, r0 + n * 4:r0 + n * 4 + 4, :]
                            .rearrange("p r w -> p (r w)"),
                            in_=fo)

    nc.compile()
    return nc


def _get_nc():
    global _CACHED_NC
    if _CACHED_NC is None:
        _CACHED_NC = _build_nc()
    return _CACHED_NC


def kernel(x, q_w, q_b, k_w, k_b, v_w, v_b, o_w, o_b,
           pos_q, pos_k, pos_v, g_q, g_k, g_v1, g_v2):
    global LAST_RESULTS
    x = np.asarray(x, dtype=np.float32)
    q_w = np.asarray(q_w, dtype=np.float32)
    k_w = np.asarray(k_w, dtype=np.float32)
    v_w = np.asarray(v_w, dtype=np.float32)
    o_w = np.asarray(o_w, dtype=np.float32)
    q_b = np.asarray(q_b, dtype=np.float32)
    k_b = np.asarray(k_b, dtype=np.float32)
    v_b = np.asarray(v_b, dtype=np.float32)
    o_b = np.asarray(o_b, dtype=np.float32)
    pq = np.asarray(pos_q, dtype=np.float32)[0, :, :W, :]  # [NH, W, HD]
    pk = np.asarray(pos_k, dtype=np.float32)[0, :, :W, :]
    pv = np.asarray(pos_v, dtype=np.float32)[0, :, :W, :]
    gq = float(np.asarray(g_q).reshape(-1)[0])
    gk = float(np.asarray(g_k).reshape(-1)[0])
    gv1 = float(np.asarray(g_v1).reshape(-1)[0])
    gv2 = float(np.asarray(g_v2).reshape(-1)[0])

    scale = HD ** (-0.5)

    # chunk-major per core: [NCH, P, RPC, W] for 4KB-contiguous DMA runs
    xt_all = x.transpose(0, 2, 1, 3).reshape(ROWS, C, W).astype(BF16)
    q_wt = np.ascontiguousarray(q_w.T * scale).astype(BF16)
    k_wt = np.ascontiguousarray(k_w.T).astype(BF16)
    v_wt = np.ascontiguousarray(v_w.T * gv1).astype(BF16)
    o_wt = np.ascontiguousarray(o_w.T).astype(BF16)

    # pos tables, chunk-major [NCH, P, W] (channel on partition axis)
    pqc = (gq * pq).transpose(0, 2, 1).reshape(NCH, P, W)       # pq'
    pkc = (gk * scale * pk).transpose(0, 2, 1).reshape(NCH, P, W)  # pk'
    pqts = np.ascontiguousarray(pqc).astype(BF16)
    mpkts = np.ascontiguousarray(-pkc).astype(BF16)

    # replicated drain tables: rep[m, p, rr*W + i] = bias[ch] + pos[ch, i]
    qb_s = (q_b * scale).reshape(NCH, P, 1)
    kb_s = k_b.reshape(NCH, P, 1)
    qpk_rep = np.ascontiguousarray(
        np.tile(pkc + qb_s, (1, 1, 512 // W))).astype(BF16)
    kpq_rep = np.ascontiguousarray(
        np.tile(pqc + kb_s, (1, 1, 512 // W))).astype(BF16)

    pvs = np.ascontiguousarray(
        gv2 * pv.transpose(1, 0, 2).reshape(W, C)
        + gv1 * v_b[None, :]).astype(BF16)
    ident = np.eye(P, dtype=np.float32).astype(BF16)

    shared = {
        "q_wt": q_wt, "k_wt": k_wt, "v_wt": v_wt, "o_wt": o_wt,
        "pqts": pqts, "mpkts": mpkts,
        "qpk_rep": qpk_rep, "kpq_rep": kpq_rep, "pvs": pvs, "ident": ident,
    }
    in_maps = []
    for c in range(NCORES):
        m = dict(shared)
        xs = xt_all[c * RPC:(c + 1) * RPC]  # [RPC, C, W]
        m["xt"] = np.ascontiguousarray(
            xs.reshape(RPC, NCH, P, W).transpose(1, 2, 0, 3))
        in_maps.append(m)

    nc = _get_nc()
    res = run_bass_kernel_spmd(nc, in_maps, core_ids=list(range(NCORES)))
    LAST_RESULTS = res

    out_all = np.concatenate(
        [res.results[c]["out_t"].transpose(2, 0, 1, 3).reshape(RPC, C, W)
         for c in range(NCORES)], axis=0)
    y = out_all.reshape(B, H, C, W).transpose(0, 2, 1, 3)
    y = y + o_b[None, :, None, None]
    return np.ascontiguousarray(y.astype(np.float32))
